# revision 30
# baseline (speedup 1.0000x reference)
# Multi-head attention (B=4, T=2048, D=1024, H=16, dqk=dv=64) on 8 trn2
# NeuronCores. Sharding: core c -> batch c//2, head-group c%2 (8 heads).
# Each core computes its batch's Q^T/K^T/V projections for its heads,
# causal flash attention with transposed scores (S^T[kv,q]; softmax
# normalizer via a ones-column appended to V), and a partial output
# projection. Host sums the two partials per batch and adds biases.
#
# The Q projection runs in fp8e4m3 DoubleRow mode (2 MACs/cell/cycle):
# weights are host-scaled by 64 so w~N(0,1.3) stays in fp8 normal range,
# making S scale by 64 which the exp scale constant absorbs. K/V and the
# output projection stay bf16: fp8 K too would push output error to 2.4%
# (over the 2e-2 gate); fp8 V/O would inject noise directly on the output.
import numpy as np
import ml_dtypes

B, TQ, TKV, DM, H, DQ, DV = 4, 2048, 2048, 1024, 16, 64, 64
NC = 8          # cores
HL = 8          # heads per core
NHP = HL // 2   # 128-partition head-pair tiles (4)
SB = 512        # q super-block width
NQSB = TQ // SB
NKT = TKV // 128
NDM = DM // 128
NDP = NDM // 2  # fp8 DoubleRow chunk-pairs (4)
P = 128

bf16 = ml_dtypes.bfloat16
f8 = ml_dtypes.float8_e4m3
WS = 64.0       # host-side fp8 weight scale for Q/K

_programs = {}
_last_in_maps = None


def _make_tc_class(tile_mod):
    from concourse.vector_clock import ScopedClock
    import concourse.mybir as mybir

    class TC(tile_mod.TileContext):
        # This toolchain's walrus codegen encodes at most ONE sync wait
        # per instruction. Tile's wait assignment can attach several, so
        # before lowering, peel extra waits off onto standalone
        # InstEventSemaphore instructions placed immediately before the
        # instruction on the same engine (in-order execution makes this
        # semantically identical).
        def _lower_ordered_insts(self, ordered):
            for bb_name, insts in ordered.items():
                out = []
                for inst in insts:
                    si = getattr(inst, "sync_info", None)
                    eng = getattr(inst, "engine", None)
                    if (
                        si is not None
                        and si.on_wait
                        and len(si.on_wait) > 1
                        and eng is not None
                        and eng != mybir.EngineType.Unassigned
                    ):
                        waits = list(si.on_wait)
                        for w in waits[:-1]:
                            ev = mybir.InstEventSemaphore(
                                name=f"I-{self.nc.next_id()}", ins=[], outs=[]
                            )
                            ev.engine = eng
                            ev.sync_info = mybir.SyncInfo(
                                on_wait=[w], on_update=[]
                            )
                            out.append(ev)
                        si.on_wait = waits[-1:]
                    out.append(inst)
                insts[:] = out
            return super()._lower_ordered_insts(ordered)

        # Same 1-wait limit applies to the tail drain; split its waits
        # into standalone wait instructions.
        def _drain_and_barrier(self, tick_clock, wait_clock):
            drain_inst = self.nc.sync.drain()
            wait_clock.add_sem_waits(
                drain_inst.ins, ScopedClock({None: tick_clock.global_clock})
            )
            si = drain_inst.ins.sync_info
            waits = list(si.on_wait) if si and si.on_wait else []
            if len(waits) > 1:
                si.on_wait = waits[:1]
                name2sem = {}
                for s in self.sems.allocated().values():
                    name2sem[getattr(s, "name", None) or str(s)] = s
                for w in waits[1:]:
                    self.nc.sync.wait_ge(name2sem[w.ant_name], w.wait_value)
            self.nc.all_engine_barrier()
            popped = self.nc._tile_sem_poison_stack.pop()
            assert popped is self._sem_poison
            self.nc.clear_and_free_semaphores(list(self.sems.allocated().values()))
            self.nc.all_engine_barrier()

    return TC


def build_program(causal: bool):
    import concourse.bass as bass
    import concourse.mybir as mybir
    import concourse.tile as tile

    dt = mybir.dt
    AF = mybir.ActivationFunctionType
    DR = mybir.MatmulPerfMode.DoubleRow
    TC = _make_tc_class(tile)

    nc = bass.Bass("TRN2", target_bir_lowering=False, debug=False, num_devices=NC)

    xqT = nc.dram_tensor("xqT", [DM, TQ], dt.float8e4, kind="ExternalInput")
    xkvT = nc.dram_tensor("xkvT", [DM, TKV], dt.bfloat16, kind="ExternalInput")
    wq_d = nc.dram_tensor("wq", [DM, HL * DQ], dt.float8e4, kind="ExternalInput")
    wk_d = nc.dram_tensor("wk", [DM, HL * DQ], dt.bfloat16, kind="ExternalInput")
    wv_d = nc.dram_tensor("wv", [DM, HL * DV], dt.bfloat16, kind="ExternalInput")
    wo_d = nc.dram_tensor("wo", [HL * DV, DM], dt.bfloat16, kind="ExternalInput")
    bq_d = nc.dram_tensor("bqp", [P, NHP], dt.float32, kind="ExternalInput")
    bk_d = nc.dram_tensor("bkp", [P, NHP], dt.float32, kind="ExternalInput")
    pad_d = nc.dram_tensor("pad", [P, NKT], dt.float32, kind="ExternalInput")
    msk_d = nc.dram_tensor("msk", [P, P], dt.bfloat16, kind="ExternalInput")
    one_d = nc.dram_tensor("one64", [P, 2 * P], dt.bfloat16, kind="ExternalInput")
    out_d = nc.dram_tensor("out", [TQ, DM], dt.float32, kind="ExternalOutput")

    # exp(scale * S' + pad): S' = (64Q)·K, true logits need /sqrt(64)
    EXP_SCALE = 0.125 / WS

    with TC(nc) as tc:
        with (
            tc.tile_pool(name="res", bufs=1) as res,
            tc.tile_pool(name="xp", bufs=8) as xp,
            tc.tile_pool(name="ptp", bufs=4) as ptp,
            tc.tile_pool(name="atp", bufs=2) as atp,
            tc.tile_pool(name="rcp", bufs=2) as rcp,
            tc.tile_pool(name="ps_proj", bufs=2, space="PSUM") as ps_proj,
            tc.tile_pool(name="ps_s", bufs=2, space="PSUM") as ps_s,
            tc.tile_pool(name="ps_at", bufs=2, space="PSUM") as ps_at,
        ):
            # ---- Q-projection critical path first: wq + bq + xqT ----
            # one trigger per tensor: each dma_start costs ~600ns of serial
            # issue time on the Sync engine, so bulk loads use a single wide
            # 3D-AP DMA with dm-chunks side by side in the free dim
            wq_t = res.tile([P, NDM * HL * DQ], dt.float8e4, tag="wq", name="wq")
            nc.sync.dma_start(
                wq_t[:].rearrange("p (k m) -> p k m", k=NDM),
                wq_d.ap().rearrange("(k p) m -> p k m", p=P))

            def wq_pair(cp, hp):
                # fp8 DoubleRow lhsT [128, ko=2, 128]: ko = second 128-row
                # half of the 256-row dm chunk-pair
                return wq_t[:, 2 * cp * HL * DQ:(2 * cp + 2) * HL * DQ].rearrange(
                    "p (ko m) -> p ko m", ko=2)[:, :, hp * P:(hp + 1) * P]

            bq_t = res.tile([P, NHP], dt.float32, tag="bq", name="bq_t")
            nc.sync.dma_start(bq_t[:], bq_d.ap()[:, :])

            # ---- Q^T projection, per q super-block (chunked xq) ----
            qT = [res.tile([P, TQ], dt.bfloat16, tag=f"qT{hp}", name=f"qT{hp}") for hp in range(NHP)]
            xq_cache = {}

            def load_xq(qsb):
                if qsb in xq_cache:
                    return xq_cache[qsb]
                t = xp.tile([P, NDM * SB], dt.float8e4, tag="xq", name="xq",
                            bufs=4)
                nc.sync.dma_start(
                    t[:].rearrange("p (k n) -> p k n", k=NDM),
                    xqT.ap()[:, qsb * SB:(qsb + 1) * SB].rearrange(
                        "(k p) n -> p k n", p=P))
                xq_cache[qsb] = t
                return t

            def q_units(qsb):
                def unit(hp):
                    ps = ps_proj.tile([P, SB], dt.float32, tag="pp", name="pp")
                    xqt = load_xq(qsb)
                    for k in range(NDP):
                        nc.tensor.matmul(
                            ps[:],
                            wq_pair(k, hp),
                            xqt[:, 2 * k * SB:(2 * k + 2) * SB].rearrange(
                                "p (ko n) -> p ko n", ko=2),
                            start=(k == 0), stop=(k == NDP - 1),
                            perf_mode=DR,
                        )
                    with nc.allow_low_precision(reason="bf16 Q"):
                        nc.vector.tensor_scalar_add(
                            qT[hp][:, qsb * SB:(qsb + 1) * SB], ps[:],
                            bq_t[:, hp:hp + 1],
                        )
                return [lambda hp=hp: unit(hp) for hp in range(NHP)]

            load_xq(0)  # qsb0 activations right behind wq on the DMA queue
            xkv_cache = {}

            def load_xkv(nj):
                if nj in xkv_cache:
                    return xkv_cache[nj]
                t = xp.tile([P, NDM * SB], dt.bfloat16, tag="xkv", name="xkv",
                            bufs=4)
                nc.sync.dma_start(
                    t[:].rearrange("p (k n) -> p k n", k=NDM),
                    xkvT.ap()[:, nj * SB:(nj + 1) * SB].rearrange(
                        "(k p) n -> p k n", p=P))
                xkv_cache[nj] = t
                return t

            # the 4MB xkv block gates the K/V projections -> trigger it
            # before the weights so its transfer leads the queue
            load_xkv(0)
            wk_t = res.tile([P, NDM * HL * DQ], dt.bfloat16, tag="wk", name="wk")
            nc.sync.dma_start(
                wk_t[:].rearrange("p (k m) -> p k m", k=NDM),
                wk_d.ap().rearrange("(k p) m -> p k m", p=P))
            wv_t = res.tile([P, NDM * HL * DV], dt.bfloat16, tag="wv", name="wv")
            nc.sync.dma_start(
                wv_t[:].rearrange("p (k m) -> p k m", k=NDM),
                wv_d.ap().rearrange("(k p) m -> p k m", p=P))
            bk_t = res.tile([P, NHP], dt.float32, tag="bk", name="bk_t")
            nc.sync.dma_start(bk_t[:], bk_d.ap()[:, :])
            pad_t = res.tile([P, NKT], dt.float32, tag="pad", name="pad_t")
            nc.sync.dma_start(pad_t[:], pad_d.ap()[:, :])
            msk_t = res.tile([P, P], dt.bfloat16, tag="msk", name="msk_t")
            nc.sync.dma_start(msk_t[:], msk_d.ap()[:, :])
            sel_t = res.tile([P, 2 * P], dt.bfloat16, tag="sel", name="sel_t")
            nc.sync.dma_start(sel_t[:], one_d.ap()[:, :])
            wo_t = res.tile([P, NHP * DM], dt.bfloat16, tag="wo", name="wo")
            nc.sync.dma_start(
                wo_t[:].rearrange("p (k m) -> p k m", k=NHP),
                wo_d.ap().rearrange("(k p) m -> p k m", p=P))

            # ---- K^T and V projections (kv-chunk major so attention can
            # start early). V layout: per kv tile [128, 8 heads x 65]
            # (64 V cols + 1 ones col per head) for the PV+rowsum matmul. ----
            kT = [res.tile([P, TKV], dt.bfloat16, tag=f"kT{hp}", name=f"kT{hp}") for hp in range(NHP)]
            v_t = [res.tile([P, HL * 65], dt.bfloat16, tag=f"v{vt}", name=f"v{vt}") for vt in range(NKT)]
            def kv_units(nj):
                def k_unit(hp):
                    ps = ps_proj.tile([P, SB], dt.float32, tag="pp", name="pp")
                    xkt = load_xkv(nj)
                    for k in range(NDM):
                        nc.tensor.matmul(
                            ps[:],
                            wk_t[:, k * HL * DQ + hp * P:k * HL * DQ + (hp + 1) * P],
                            xkt[:, k * SB:(k + 1) * SB],
                            start=(k == 0), stop=(k == NDM - 1),
                        )
                    with nc.allow_low_precision(reason="bf16 K"):
                        nc.vector.tensor_scalar_add(
                            kT[hp][:, nj * SB:(nj + 1) * SB], ps[:],
                            bk_t[:, hp:hp + 1],
                        )

                def v_unit(vt):
                    ps = ps_proj.tile([P, SB], dt.float32, tag="pp", name="pp")
                    xkt = load_xkv(vt // 4)
                    for k in range(NDM):
                        nc.tensor.matmul(
                            ps[:],
                            xkt[:, k * SB + (vt % 4) * P:k * SB + (vt % 4 + 1) * P],
                            wv_t[:, k * HL * DV:(k + 1) * HL * DV],
                            start=(k == 0), stop=(k == NDM - 1),
                        )
                    vtile = v_t[vt]
                    nc.vector.tensor_copy(
                        vtile[:].rearrange("p (h d) -> p h d", d=65)[:, :, 0:64],
                        ps[:].rearrange("p (h d) -> p h d", d=64),
                    )
                    nc.gpsimd.memset(
                        vtile[:].rearrange("p (h d) -> p h d", d=65)[:, :, 64:65], 1.0
                    )
                units = [lambda hp=hp: k_unit(hp) for hp in range(NHP)]
                units += [lambda vt=vt: v_unit(vt) for vt in range(nj * 4, nj * 4 + 4)]
                return units

            # ---- attention + output projection, per q super-block ----
            def finalize_unit(at_tiles, an_tiles, rc_ap, hp):
                # broadcast the pair's recip rows across 2x64 partitions
                # via PE: sel variant (hp%2) places head 2hp's recip on
                # rows 0-63 and head 2hp+1's on rows 64-127
                bc = ps_proj.tile([P, SB], dt.float32, tag="pp", name="bc")
                v = hp % 2
                nc.tensor.matmul(
                    bc[:], sel_t[:, v * P:(v + 1) * P],
                    rc_ap,
                    start=True, stop=True,
                )
                nc.vector.tensor_mul(
                    at_tiles[hp][0:64, :], an_tiles[2 * hp][0:64, :],
                    bc[0:64, :]
                )
                nc.vector.tensor_mul(
                    at_tiles[hp][64:128, :], an_tiles[2 * hp + 1][0:64, :],
                    bc[64:128, :]
                )

            def outproj_unit(qsb, at_tiles, qt, col):
                ps = ps_proj.tile([P, SB], dt.float32, tag="pp", name="pp")
                for hp in range(NHP):
                    nc.tensor.matmul(
                        ps[:],
                        at_tiles[hp][:, qt * P:(qt + 1) * P],
                        wo_t[:, hp * DM + col * SB:hp * DM + (col + 1) * SB],
                        start=(hp == 0), stop=(hp == NHP - 1),
                    )
                ost = rcp.tile([P, SB], dt.float32, tag="ost", name="ost", bufs=3)
                nc.vector.tensor_copy(ost[:], ps[:])
                r0 = qsb * SB + qt * P
                nc.sync.dma_start(
                    out_d.ap()[r0:r0 + P, col * SB:(col + 1) * SB], ost[:]
                )

            def blk_units(qsb, at_tiles, an_tiles, rc):
                # finalize + output projection for a finished q block, as
                # interleavable units (finalize first: outproj reads at_tiles)
                units = [
                    lambda hp=hp: finalize_unit(
                        at_tiles, an_tiles,
                        rc[:, (hp // 2) * SB:(hp // 2 + 1) * SB], hp)
                    for hp in range(NHP)
                ]
                units += [
                    lambda qt=qt, col=col: outproj_unit(qsb, at_tiles, qt, col)
                    for qt in range(4) for col in range(2)
                ]
                return units

            prev_blk = None
            pending = q_units(0) + kv_units(0)
            for qsb in range(NQSB):
                # flush any units for THIS block not yet emitted
                for u in pending:
                    u()
                # next block's projection units AND the previous block's
                # finalize+outproj get interleaved between this block's head
                # pairs so the in-order PE stream always has independent
                # matmul work while ACT chews on exps
                pending = (
                    q_units(qsb + 1) + kv_units(qsb + 1)
                    if qsb + 1 < NQSB else []
                )
                if prev_blk is not None:
                    pending = pending + blk_units(*prev_blk)
                at_tiles = [
                    atp.tile([P, SB], dt.bfloat16, tag=f"attnT{hp}", name=f"attnT{hp}")
                    for hp in range(NHP)
                ]
                kt_max = 4 * qsb + 4 if causal else NKT
                # normalizer rows gathered at partitions {0,32,64,96} x 2
                # column groups (DVE partition offsets must be 32-aligned);
                # filler 1.0 keeps the reciprocal finite on unused rows
                sums = rcp.tile([P, 2 * SB], dt.bfloat16, tag="sums", name="sums")
                nc.gpsimd.memset(sums[:], 1.0)
                an_tiles = []

                def build_hp(hp):
                    # the pair's S matmuls target disjoint PE row groups
                    # (0-63 / 64-127) and disjoint banks of one wide PSUM
                    # tile, so they run concurrently and one wide exp
                    # covers both heads
                    st = {"aps2": None, "pv_q": []}

                    def emit_pv(kt, pt, c0):
                        if st["aps2"] is None:
                            # lazy: allocated at first PV emission, which the
                            # schedule places after the previous pair's drain
                            st["aps2"] = [
                                ps_at.tile([65, SB], dt.float32, tag="at",
                                           name="at")
                                for _ in range(2)
                            ]
                        for e in range(2):
                            h = 2 * hp + e
                            nc.tensor.matmul(
                                st["aps2"][e][:, c0:SB],
                                v_t[kt][:, h * 65:h * 65 + 65],
                                pt[:, e * SB + c0:(e + 1) * SB],
                                start=(kt == 0), stop=(kt == kt_max - 1),
                            )

                    # PV for tile kt is emitted AFTER tile kt+2's S matmuls,
                    # so the in-order PE queue never waits on exp(kt): two
                    # more S pairs run while ACT produces pt(kt)
                    def kt_step(kt):
                        # diagonal blocks (j>=1): only q columns >= 128*j can
                        # be unmasked -> trim the left columns entirely
                        j = kt - 4 * qsb if causal else -1
                        c0 = 128 * j if j > 0 else 0
                        sps = ps_s.tile([P, 2 * SB], dt.float32, tag="s",
                                        name="s")
                        for e in range(2):
                            off = e * 64
                            nc.tensor.matmul(
                                sps[:, e * SB + c0:(e + 1) * SB],
                                kT[hp][off:off + 64, kt * P:(kt + 1) * P],
                                qT[hp][off:off + 64,
                                       qsb * SB + c0:(qsb + 1) * SB],
                                start=True, stop=True,
                            )
                        pt = ptp.tile([P, 2 * SB], dt.bfloat16, tag="pT",
                                      name="pT")
                        nc.scalar.activation(
                            pt[:].rearrange("p (e q) -> p e q", e=2)[:, :, c0:SB],
                            sps[:].rearrange("p (e q) -> p e q", e=2)[:, :, c0:SB],
                            AF.Exp,
                            scale=EXP_SCALE, bias=pad_t[:, kt:kt + 1],
                        )
                        if causal and j >= 0:
                            # only the 128-wide transition strip needs the
                            # triangle; columns beyond it are fully unmasked
                            for e in range(2):
                                nc.vector.tensor_mul(
                                    pt[:, e * SB + c0:e * SB + c0 + P],
                                    pt[:, e * SB + c0:e * SB + c0 + P],
                                    msk_t[:],
                                )
                        st["pv_q"].append((kt, pt, c0))
                        if len(st["pv_q"]) > 2:
                            emit_pv(*st["pv_q"].pop(0))
                        # drip one boundary-work unit into the kt stream every
                        # few steps: the S+PV pair leaves ~200ns of PE slack
                        # per ACT-paced step, so a small chain here converts
                        # head-pair-boundary burst time into overlapped time
                        if kt % 4 == 3 and kt + 1 < kt_max and pending:
                            pending.pop(0)()

                    def pv_flush():
                        for args in st["pv_q"]:
                            emit_pv(*args)
                        st["pv_q"] = []

                    def an_drain():
                        for e in range(2):
                            h = 2 * hp + e
                            # stage numerator + normalizer row to SBUF, free
                            # psum; the normalizer row rides along and is
                            # gathered into the sums tile by a DMA engine
                            an = rcp.tile([65, SB], dt.bfloat16, tag=f"an{h}",
                                          name=f"an{h}")
                            nc.vector.tensor_copy(an[:], st["aps2"][e][0:65, :])
                            r, cg = 32 * (h % 4), (h // 4) * SB
                            nc.sync.dma_start(
                                sums[r:r + 1, cg:cg + SB], an[64:65, :]
                            )
                            an_tiles.append(an)

                    steps = [lambda kt=kt: kt_step(kt) for kt in range(kt_max)]
                    return steps, pv_flush, an_drain

                # zippered schedule across head pairs: before pair hp's
                # drain, pair hp+1's first LA S/exp steps are emitted so ACT
                # stays fed through the boundary while PE does the drain,
                # pops, and finalize work
                LA = 2
                hps = [build_hp(hp) for hp in range(NHP)]
                for hp in range(NHP):
                    steps, pv_flush, an_drain = hps[hp]
                    for s in steps[LA if hp > 0 else 0:]:
                        s()
                    pv_flush()
                    if hp + 1 < NHP:
                        for s in hps[hp + 1][0][:LA]:
                            s()
                    an_drain()
                    for _ in range(5):
                        if pending:
                            pending.pop(0)()
                    if qsb == NQSB - 1 and hp in (1, 3):
                        # last q-block: finalize each 4-head column group as
                        # soon as its normalizers are gathered, so only half
                        # the finalize chain sits on the kernel tail
                        half = hp // 2
                        lnh = rcp.tile([P, SB], dt.float32, tag="lnh",
                                       name="lnh")
                        nc.scalar.activation(
                            lnh[:], sums[:, half * SB:(half + 1) * SB], AF.Ln)
                        rch = rcp.tile([P, SB], dt.bfloat16, tag="rch",
                                       name="rch")
                        with nc.allow_low_precision(reason="bf16 recip"):
                            nc.scalar.activation(rch[:], lnh[:], AF.Exp,
                                                 scale=-1.0)
                        finalize_unit(at_tiles, an_tiles, rch[:], 2 * half)
                        finalize_unit(at_tiles, an_tiles, rch[:], 2 * half + 1)
                if qsb < NQSB - 1:
                    # batched reciprocal for all 8 heads of this q-block, as
                    # exp(-ln(x)) on ACT: both functions live in the
                    # natural_log_exp_and_others table set, so no table
                    # reloads, and it's ~10x cheaper than DVE reciprocal
                    lns = rcp.tile([P, 2 * SB], dt.float32, tag="lns", name="lns")
                    nc.scalar.activation(lns[:], sums[:], AF.Ln)
                    rc = rcp.tile([P, 2 * SB], dt.bfloat16, tag="rc", name="rc")
                    with nc.allow_low_precision(reason="bf16 softmax recip"):
                        nc.scalar.activation(rc[:], lns[:], AF.Exp, scale=-1.0)
                    prev_blk = (qsb, at_tiles, an_tiles, rc)
                else:
                    last_at = at_tiles
            # drain: leftover units of the second-to-last block, then the
            # last block's output projection (its finalize ran per-colgroup)
            for u in pending:
                u()
            for qt in range(4):
                for col in range(2):
                    outproj_unit(NQSB - 1, last_at, qt, col)
    return nc


def _get_program(causal: bool):
    key = bool(causal)
    if key not in _programs:
        _programs[key] = build_program(key)
    return _programs[key]


def kernel(**inputs):
    from concourse.bass_utils import run_bass_kernel_spmd

    xq = np.asarray(inputs["query_sequence"], dtype=np.float32)
    xkv = np.asarray(inputs["key_value_sequence"], dtype=np.float32)
    pmask = np.asarray(inputs["key_value_padding_mask"])
    Wq = np.asarray(inputs["Wq"], dtype=np.float32)
    bq = np.asarray(inputs["bq"], dtype=np.float32)
    Wkv = np.asarray(inputs["Wkv"], dtype=np.float32)
    bkv = np.asarray(inputs["bkv"], dtype=np.float32)
    Wo = np.asarray(inputs["Wo"], dtype=np.float32)
    bo = np.asarray(inputs["bo"], dtype=np.float32)
    causal = bool(np.asarray(inputs["apply_causal_mask"]))

    nc = _get_program(causal)

    Wk_full = Wkv[:, : H * DQ]
    Wv_full = Wkv[:, H * DQ:]
    bk_full = bkv[: H * DQ]
    bv_full = bkv[H * DQ:]

    # causal transition-strip triangle: tri[kv, q'] = 1 if q' >= kv, for the
    # single 128-wide strip of each diagonal block
    kvi = np.arange(P)[:, None]
    qi = np.arange(P)[None, :]
    msk = (qi >= kvi).astype(np.float32).astype(bf16)
    # recip-broadcast selectors, two heads per matmul:
    # variant A (cols 0-127): rows 0/32, variant B (cols 128-255): rows 64/96
    sel = np.zeros((P, 2 * P), np.float32)
    sel[0, 0:64] = 1.0
    sel[32, 64:128] = 1.0
    sel[64, 128:192] = 1.0
    sel[96, 192:256] = 1.0
    sel = sel.astype(bf16)

    in_maps = []
    for c in range(NC):
        b, g = divmod(c, 2)
        hs = slice(g * HL * DQ, (g + 1) * HL * DQ)
        pb = np.where(pmask[b], np.float32(-1e30), np.float32(0.0))
        in_maps.append({
            "xqT": np.ascontiguousarray(xq[b].T).astype(f8),
            "xkvT": np.ascontiguousarray(xkv[b].T).astype(bf16),
            "wq": np.ascontiguousarray(Wq[:, hs] * WS).astype(f8),
            "wk": np.ascontiguousarray(Wk_full[:, hs]).astype(bf16),
            "wv": np.ascontiguousarray(Wv_full[:, hs]).astype(bf16),
            "wo": np.ascontiguousarray(Wo[hs, :]).astype(bf16),
            "bqp": np.ascontiguousarray((bq[hs] * WS).reshape(NHP, P).T),
            "bkp": np.ascontiguousarray(bk_full[hs].reshape(NHP, P).T),
            "pad": np.ascontiguousarray(pb.reshape(NKT, P).T),
            "msk": msk,
            "one64": sel,
        })

    global _last_in_maps
    _last_in_maps = in_maps
    res = run_bass_kernel_spmd(nc, in_maps, core_ids=list(range(NC)))

    host_bias = bo + bv_full @ Wo  # softmax rows sum to 1 -> V-bias is additive
    out = np.empty((B, TQ, DM), np.float32)
    for b in range(B):
        out[b] = res.results[2 * b]["out"] + res.results[2 * b + 1]["out"] + host_bias
    return out


# revision 34
# speedup vs baseline: 1.0139x; 1.0139x over previous
# Multi-head attention (B=4, T=2048, D=1024, H=16, dqk=dv=64) on 8 trn2
# NeuronCores. Sharding: core c -> batch c//2, head-group c%2 (8 heads).
# Each core computes its batch's Q^T/K^T/V projections for its heads,
# causal flash attention with transposed scores (S^T[kv,q]; softmax
# normalizer via a ones-column appended to V), and a partial output
# projection. Host sums the two partials per batch and adds biases.
#
# The Q projection runs in fp8e4m3 DoubleRow mode (2 MACs/cell/cycle):
# weights are host-scaled by 64 so w~N(0,1.3) stays in fp8 normal range,
# making S scale by 64 which the exp scale constant absorbs. K/V and the
# output projection stay bf16: fp8 K too would push output error to 2.4%
# (over the 2e-2 gate); fp8 V/O would inject noise directly on the output.
import numpy as np
import ml_dtypes

B, TQ, TKV, DM, H, DQ, DV = 4, 2048, 2048, 1024, 16, 64, 64
NC = 8          # cores
HL = 8          # heads per core
NHP = HL // 2   # 128-partition head-pair tiles (4)
SB = 512        # q super-block width
NQSB = TQ // SB
NKT = TKV // 128
NDM = DM // 128
NDP = NDM // 2  # fp8 DoubleRow chunk-pairs (4)
P = 128

bf16 = ml_dtypes.bfloat16
f8 = ml_dtypes.float8_e4m3
WS = 64.0       # host-side fp8 weight scale for Q/K

_programs = {}
_last_in_maps = None


def _make_tc_class(tile_mod):
    from concourse.vector_clock import ScopedClock
    import concourse.mybir as mybir

    class TC(tile_mod.TileContext):
        # This toolchain's walrus codegen encodes at most ONE sync wait
        # per instruction. Tile's wait assignment can attach several, so
        # before lowering, peel extra waits off onto standalone
        # InstEventSemaphore instructions placed immediately before the
        # instruction on the same engine (in-order execution makes this
        # semantically identical).
        def _lower_ordered_insts(self, ordered):
            for bb_name, insts in ordered.items():
                out = []
                for inst in insts:
                    si = getattr(inst, "sync_info", None)
                    eng = getattr(inst, "engine", None)
                    if (
                        si is not None
                        and si.on_wait
                        and len(si.on_wait) > 1
                        and eng is not None
                        and eng != mybir.EngineType.Unassigned
                    ):
                        waits = list(si.on_wait)
                        for w in waits[:-1]:
                            ev = mybir.InstEventSemaphore(
                                name=f"I-{self.nc.next_id()}", ins=[], outs=[]
                            )
                            ev.engine = eng
                            ev.sync_info = mybir.SyncInfo(
                                on_wait=[w], on_update=[]
                            )
                            out.append(ev)
                        si.on_wait = waits[-1:]
                    out.append(inst)
                insts[:] = out
            return super()._lower_ordered_insts(ordered)

        # Same 1-wait limit applies to the tail drain; split its waits
        # into standalone wait instructions.
        def _drain_and_barrier(self, tick_clock, wait_clock):
            drain_inst = self.nc.sync.drain()
            wait_clock.add_sem_waits(
                drain_inst.ins, ScopedClock({None: tick_clock.global_clock})
            )
            si = drain_inst.ins.sync_info
            waits = list(si.on_wait) if si and si.on_wait else []
            if len(waits) > 1:
                si.on_wait = waits[:1]
                name2sem = {}
                for s in self.sems.allocated().values():
                    name2sem[getattr(s, "name", None) or str(s)] = s
                for w in waits[1:]:
                    self.nc.sync.wait_ge(name2sem[w.ant_name], w.wait_value)
            self.nc.all_engine_barrier()
            popped = self.nc._tile_sem_poison_stack.pop()
            assert popped is self._sem_poison
            self.nc.clear_and_free_semaphores(list(self.sems.allocated().values()))
            self.nc.all_engine_barrier()

    return TC


def build_program(causal: bool):
    import concourse.bass as bass
    import concourse.mybir as mybir
    import concourse.tile as tile

    dt = mybir.dt
    AF = mybir.ActivationFunctionType
    DR = mybir.MatmulPerfMode.DoubleRow
    TC = _make_tc_class(tile)

    nc = bass.Bass("TRN2", target_bir_lowering=False, debug=False, num_devices=NC)

    xqT = nc.dram_tensor("xqT", [DM, TQ], dt.float8e4, kind="ExternalInput")
    xkvT = nc.dram_tensor("xkvT", [DM, TKV], dt.bfloat16, kind="ExternalInput")
    wq_d = nc.dram_tensor("wq", [DM, HL * DQ], dt.float8e4, kind="ExternalInput")
    wk_d = nc.dram_tensor("wk", [DM, HL * DQ], dt.bfloat16, kind="ExternalInput")
    wv_d = nc.dram_tensor("wv", [DM, HL * DV], dt.bfloat16, kind="ExternalInput")
    wo_d = nc.dram_tensor("wo", [HL * DV, DM], dt.bfloat16, kind="ExternalInput")
    bq_d = nc.dram_tensor("bqp", [P, NHP], dt.float32, kind="ExternalInput")
    bk_d = nc.dram_tensor("bkp", [P, NHP], dt.float32, kind="ExternalInput")
    pad_d = nc.dram_tensor("pad", [P, NKT], dt.float32, kind="ExternalInput")
    msk_d = nc.dram_tensor("msk", [P, P], dt.bfloat16, kind="ExternalInput")
    one_d = nc.dram_tensor("one64", [P, 2 * P], dt.bfloat16, kind="ExternalInput")
    out_d = nc.dram_tensor("out", [TQ, DM], dt.float32, kind="ExternalOutput")

    # exp(scale * S' + pad): S' = (64Q)·K, true logits need /sqrt(64)
    EXP_SCALE = 0.125 / WS

    with TC(nc) as tc:
        with (
            tc.tile_pool(name="res", bufs=1) as res,
            tc.tile_pool(name="xp", bufs=8) as xp,
            tc.tile_pool(name="ptp", bufs=4) as ptp,
            tc.tile_pool(name="atp", bufs=2) as atp,
            tc.tile_pool(name="rcp", bufs=2) as rcp,
            tc.tile_pool(name="ps_proj", bufs=2, space="PSUM") as ps_proj,
            tc.tile_pool(name="ps_s", bufs=2, space="PSUM") as ps_s,
            tc.tile_pool(name="ps_at", bufs=2, space="PSUM") as ps_at,
        ):
            # ---- Q-projection critical path first: wq + bq + xqT ----
            # one trigger per tensor: each dma_start costs ~600ns of serial
            # issue time on the Sync engine, so bulk loads use a single wide
            # 3D-AP DMA with dm-chunks side by side in the free dim
            # wq in two halves so the first Q matmul starts after half the
            # transfer (it only waits on the chunks it reads)
            wq_t = res.tile([P, NDM * HL * DQ], dt.float8e4, tag="wq", name="wq")
            H4 = NDM // 2
            for h4 in range(2):
                nc.sync.dma_start(
                    wq_t[:, h4 * H4 * HL * DQ:(h4 + 1) * H4 * HL * DQ].rearrange(
                        "p (k m) -> p k m", k=H4),
                    wq_d.ap()[h4 * H4 * P:(h4 + 1) * H4 * P, :].rearrange(
                        "(k p) m -> p k m", p=P))

            def wq_pair(cp, hp):
                # fp8 DoubleRow lhsT [128, ko=2, 128]: ko = second 128-row
                # half of the 256-row dm chunk-pair
                return wq_t[:, 2 * cp * HL * DQ:(2 * cp + 2) * HL * DQ].rearrange(
                    "p (ko m) -> p ko m", ko=2)[:, :, hp * P:(hp + 1) * P]

            bq_t = res.tile([P, NHP], dt.float32, tag="bq", name="bq_t")
            nc.sync.dma_start(bq_t[:], bq_d.ap()[:, :])

            # ---- Q^T projection, per q super-block (chunked xq) ----
            qT = [res.tile([P, TQ], dt.bfloat16, tag=f"qT{hp}", name=f"qT{hp}") for hp in range(NHP)]
            xq_cache = {}

            def load_xq(qsb):
                if qsb in xq_cache:
                    return xq_cache[qsb]
                t = xp.tile([P, NDM * SB], dt.float8e4, tag="xq", name="xq",
                            bufs=4)
                # first block in two halves (ramp: Q matmuls start sooner);
                # later blocks as one trigger
                nsp = 2 if qsb == 0 else 1
                ck = NDM // nsp
                for sp in range(nsp):
                    nc.sync.dma_start(
                        t[:, sp * ck * SB:(sp + 1) * ck * SB].rearrange(
                            "p (k n) -> p k n", k=ck),
                        xqT.ap()[sp * ck * P:(sp + 1) * ck * P,
                                 qsb * SB:(qsb + 1) * SB].rearrange(
                            "(k p) n -> p k n", p=P))
                xq_cache[qsb] = t
                return t

            def q_units(qsb):
                def unit(hp):
                    ps = ps_proj.tile([P, SB], dt.float32, tag="pp", name="pp")
                    xqt = load_xq(qsb)
                    for k in range(NDP):
                        nc.tensor.matmul(
                            ps[:],
                            wq_pair(k, hp),
                            xqt[:, 2 * k * SB:(2 * k + 2) * SB].rearrange(
                                "p (ko n) -> p ko n", ko=2),
                            start=(k == 0), stop=(k == NDP - 1),
                            perf_mode=DR,
                        )
                    with nc.allow_low_precision(reason="bf16 Q"):
                        nc.vector.tensor_scalar_add(
                            qT[hp][:, qsb * SB:(qsb + 1) * SB], ps[:],
                            bq_t[:, hp:hp + 1],
                        )
                return [lambda hp=hp: unit(hp) for hp in range(NHP)]

            load_xq(0)  # qsb0 activations right behind wq on the DMA queue
            xkv_cache = {}

            def load_xkv(nj):
                if nj in xkv_cache:
                    return xkv_cache[nj]
                t = xp.tile([P, NDM * SB], dt.bfloat16, tag="xkv", name="xkv",
                            bufs=4)
                nc.sync.dma_start(
                    t[:].rearrange("p (k n) -> p k n", k=NDM),
                    xkvT.ap()[:, nj * SB:(nj + 1) * SB].rearrange(
                        "(k p) n -> p k n", p=P))
                xkv_cache[nj] = t
                return t

            # the 4MB xkv block gates the K/V projections -> trigger it
            # before the weights so its transfer leads the queue
            load_xkv(0)
            wk_t = res.tile([P, NDM * HL * DQ], dt.bfloat16, tag="wk", name="wk")
            nc.sync.dma_start(
                wk_t[:].rearrange("p (k m) -> p k m", k=NDM),
                wk_d.ap().rearrange("(k p) m -> p k m", p=P))
            wv_t = res.tile([P, NDM * HL * DV], dt.bfloat16, tag="wv", name="wv")
            nc.sync.dma_start(
                wv_t[:].rearrange("p (k m) -> p k m", k=NDM),
                wv_d.ap().rearrange("(k p) m -> p k m", p=P))
            bk_t = res.tile([P, NHP], dt.float32, tag="bk", name="bk_t")
            nc.sync.dma_start(bk_t[:], bk_d.ap()[:, :])
            pad_t = res.tile([P, NKT], dt.float32, tag="pad", name="pad_t")
            nc.sync.dma_start(pad_t[:], pad_d.ap()[:, :])
            msk_t = res.tile([P, P], dt.bfloat16, tag="msk", name="msk_t")
            nc.sync.dma_start(msk_t[:], msk_d.ap()[:, :])
            sel_t = res.tile([P, 2 * P], dt.bfloat16, tag="sel", name="sel_t")
            nc.sync.dma_start(sel_t[:], one_d.ap()[:, :])
            wo_t = res.tile([P, NHP * DM], dt.bfloat16, tag="wo", name="wo")
            nc.sync.dma_start(
                wo_t[:].rearrange("p (k m) -> p k m", k=NHP),
                wo_d.ap().rearrange("(k p) m -> p k m", p=P))

            # ---- K^T and V projections (kv-chunk major so attention can
            # start early). V layout: per kv tile [128, 8 heads x 65]
            # (64 V cols + 1 ones col per head) for the PV+rowsum matmul. ----
            kT = [res.tile([P, TKV], dt.bfloat16, tag=f"kT{hp}", name=f"kT{hp}") for hp in range(NHP)]
            v_t = [res.tile([P, HL * 65], dt.bfloat16, tag=f"v{vt}", name=f"v{vt}") for vt in range(NKT)]
            def kv_units(nj):
                def k_unit(hp):
                    ps = ps_proj.tile([P, SB], dt.float32, tag="pp", name="pp")
                    xkt = load_xkv(nj)
                    for k in range(NDM):
                        nc.tensor.matmul(
                            ps[:],
                            wk_t[:, k * HL * DQ + hp * P:k * HL * DQ + (hp + 1) * P],
                            xkt[:, k * SB:(k + 1) * SB],
                            start=(k == 0), stop=(k == NDM - 1),
                        )
                    with nc.allow_low_precision(reason="bf16 K"):
                        nc.vector.tensor_scalar_add(
                            kT[hp][:, nj * SB:(nj + 1) * SB], ps[:],
                            bk_t[:, hp:hp + 1],
                        )

                def v_unit(vt):
                    ps = ps_proj.tile([P, SB], dt.float32, tag="pp", name="pp")
                    xkt = load_xkv(vt // 4)
                    for k in range(NDM):
                        nc.tensor.matmul(
                            ps[:],
                            xkt[:, k * SB + (vt % 4) * P:k * SB + (vt % 4 + 1) * P],
                            wv_t[:, k * HL * DV:(k + 1) * HL * DV],
                            start=(k == 0), stop=(k == NDM - 1),
                        )
                    vtile = v_t[vt]
                    nc.vector.tensor_copy(
                        vtile[:].rearrange("p (h d) -> p h d", d=65)[:, :, 0:64],
                        ps[:].rearrange("p (h d) -> p h d", d=64),
                    )
                    nc.gpsimd.memset(
                        vtile[:].rearrange("p (h d) -> p h d", d=65)[:, :, 64:65], 1.0
                    )
                units = [lambda hp=hp: k_unit(hp) for hp in range(NHP)]
                units += [lambda vt=vt: v_unit(vt) for vt in range(nj * 4, nj * 4 + 4)]
                return units

            # ---- attention + output projection, per q super-block ----
            def finalize_unit(at_tiles, an_tiles, rc_ap, hp):
                # broadcast the pair's recip rows across 2x64 partitions
                # via PE: sel variant (hp%2) places head 2hp's recip on
                # rows 0-63 and head 2hp+1's on rows 64-127
                bc = ps_proj.tile([P, SB], dt.float32, tag="pp", name="bc")
                v = hp % 2
                nc.tensor.matmul(
                    bc[:], sel_t[:, v * P:(v + 1) * P],
                    rc_ap,
                    start=True, stop=True,
                )
                nc.vector.tensor_mul(
                    at_tiles[hp][0:64, :], an_tiles[2 * hp][0:64, :],
                    bc[0:64, :]
                )
                nc.vector.tensor_mul(
                    at_tiles[hp][64:128, :], an_tiles[2 * hp + 1][0:64, :],
                    bc[64:128, :]
                )

            def outproj_unit(qsb, at_tiles, qt, col):
                ps = ps_proj.tile([P, SB], dt.float32, tag="pp", name="pp")
                for hp in range(NHP):
                    nc.tensor.matmul(
                        ps[:],
                        at_tiles[hp][:, qt * P:(qt + 1) * P],
                        wo_t[:, hp * DM + col * SB:hp * DM + (col + 1) * SB],
                        start=(hp == 0), stop=(hp == NHP - 1),
                    )
                ost = rcp.tile([P, SB], dt.float32, tag="ost", name="ost", bufs=3)
                nc.vector.tensor_copy(ost[:], ps[:])
                r0 = qsb * SB + qt * P
                nc.sync.dma_start(
                    out_d.ap()[r0:r0 + P, col * SB:(col + 1) * SB], ost[:]
                )

            def blk_units(qsb, at_tiles, an_tiles, rc):
                # finalize + output projection for a finished q block, as
                # interleavable units (finalize first: outproj reads at_tiles)
                units = [
                    lambda hp=hp: finalize_unit(
                        at_tiles, an_tiles,
                        rc[:, (hp // 2) * SB:(hp // 2 + 1) * SB], hp)
                    for hp in range(NHP)
                ]
                units += [
                    lambda qt=qt, col=col: outproj_unit(qsb, at_tiles, qt, col)
                    for qt in range(4) for col in range(2)
                ]
                return units

            prev_blk = None
            pending = q_units(0) + kv_units(0)
            for qsb in range(NQSB):
                # flush any units for THIS block not yet emitted
                for u in pending:
                    u()
                # next block's projection units AND the previous block's
                # finalize+outproj get interleaved between this block's head
                # pairs so the in-order PE stream always has independent
                # matmul work while ACT chews on exps
                pending = (
                    q_units(qsb + 1) + kv_units(qsb + 1)
                    if qsb + 1 < NQSB else []
                )
                if prev_blk is not None:
                    pending = pending + blk_units(*prev_blk)
                at_tiles = [
                    atp.tile([P, SB], dt.bfloat16, tag=f"attnT{hp}", name=f"attnT{hp}")
                    for hp in range(NHP)
                ]
                kt_max = 4 * qsb + 4 if causal else NKT
                # normalizer rows gathered at partitions {0,32,64,96} x 2
                # column groups (DVE partition offsets must be 32-aligned);
                # filler 1.0 keeps the reciprocal finite on unused rows
                sums = rcp.tile([P, 2 * SB], dt.bfloat16, tag="sums", name="sums")
                nc.gpsimd.memset(sums[:], 1.0)
                an_tiles = []

                def build_hp(hp):
                    # the pair's S matmuls target disjoint PE row groups
                    # (0-63 / 64-127) and disjoint banks of one wide PSUM
                    # tile, so they run concurrently and one wide exp
                    # covers both heads
                    st = {"aps2": None, "pv_q": []}

                    def emit_pv(kt, pt, c0):
                        if st["aps2"] is None:
                            # lazy: allocated at first PV emission, which the
                            # schedule places after the previous pair's drain
                            st["aps2"] = [
                                ps_at.tile([65, SB], dt.float32, tag="at",
                                           name="at")
                                for _ in range(2)
                            ]
                        for e in range(2):
                            h = 2 * hp + e
                            nc.tensor.matmul(
                                st["aps2"][e][:, c0:SB],
                                v_t[kt][:, h * 65:h * 65 + 65],
                                pt[:, e * SB + c0:(e + 1) * SB],
                                start=(kt == 0), stop=(kt == kt_max - 1),
                            )

                    # PV for tile kt is emitted AFTER tile kt+2's S matmuls,
                    # so the in-order PE queue never waits on exp(kt): two
                    # more S pairs run while ACT produces pt(kt)
                    def kt_step(kt):
                        # diagonal blocks (j>=1): only q columns >= 128*j can
                        # be unmasked -> trim the left columns entirely
                        j = kt - 4 * qsb if causal else -1
                        c0 = 128 * j if j > 0 else 0
                        sps = ps_s.tile([P, 2 * SB], dt.float32, tag="s",
                                        name="s")
                        for e in range(2):
                            off = e * 64
                            nc.tensor.matmul(
                                sps[:, e * SB + c0:(e + 1) * SB],
                                kT[hp][off:off + 64, kt * P:(kt + 1) * P],
                                qT[hp][off:off + 64,
                                       qsb * SB + c0:(qsb + 1) * SB],
                                start=True, stop=True,
                            )
                        pt = ptp.tile([P, 2 * SB], dt.bfloat16, tag="pT",
                                      name="pT")
                        nc.scalar.activation(
                            pt[:].rearrange("p (e q) -> p e q", e=2)[:, :, c0:SB],
                            sps[:].rearrange("p (e q) -> p e q", e=2)[:, :, c0:SB],
                            AF.Exp,
                            scale=EXP_SCALE, bias=pad_t[:, kt:kt + 1],
                        )
                        if causal and j >= 0:
                            # only the 128-wide transition strip needs the
                            # triangle; columns beyond it are fully unmasked
                            for e in range(2):
                                nc.vector.tensor_mul(
                                    pt[:, e * SB + c0:e * SB + c0 + P],
                                    pt[:, e * SB + c0:e * SB + c0 + P],
                                    msk_t[:],
                                )
                        st["pv_q"].append((kt, pt, c0))
                        if len(st["pv_q"]) > 2:
                            emit_pv(*st["pv_q"].pop(0))
                        # drip one boundary-work unit into the kt stream every
                        # few steps: the S+PV pair leaves ~200ns of PE slack
                        # per ACT-paced step, so a small chain here converts
                        # head-pair-boundary burst time into overlapped time
                        if kt % 4 == 3 and kt + 1 < kt_max and pending:
                            pending.pop(0)()

                    def pv_flush():
                        for args in st["pv_q"]:
                            emit_pv(*args)
                        st["pv_q"] = []

                    def an_drain():
                        for e in range(2):
                            h = 2 * hp + e
                            # stage numerator + normalizer row to SBUF, free
                            # psum; the normalizer row rides along and is
                            # gathered into the sums tile by a DMA engine --
                            # except the very last pair, whose gather sits on
                            # the kernel tail: DVE beats the ~1.7us DMA
                            # launch latency there
                            an = rcp.tile([65, SB], dt.bfloat16, tag=f"an{h}",
                                          name=f"an{h}")
                            nc.vector.tensor_copy(an[:], st["aps2"][e][0:65, :])
                            r, cg = 32 * (h % 4), (h // 4) * SB
                            if qsb == NQSB - 1 and hp == NHP - 1:
                                nc.vector.tensor_copy(
                                    sums[r:r + 1, cg:cg + SB], an[64:65, :]
                                )
                            else:
                                nc.sync.dma_start(
                                    sums[r:r + 1, cg:cg + SB], an[64:65, :]
                                )
                            an_tiles.append(an)

                    steps = [lambda kt=kt: kt_step(kt) for kt in range(kt_max)]
                    return steps, pv_flush, an_drain

                # measured: emitting the next pair's first S/exp steps before
                # this pair's drain (cross-pair lookahead) ran slower — the
                # next pair's S stalls on PSUM buffers and blocks the ready
                # drain work behind it in the in-order queue. Keep LA=0.
                LA = 0
                hps = [build_hp(hp) for hp in range(NHP)]
                for hp in range(NHP):
                    steps, pv_flush, an_drain = hps[hp]
                    for s in steps[LA if hp > 0 else 0:]:
                        s()
                    pv_flush()
                    if LA and hp + 1 < NHP:
                        for s in hps[hp + 1][0][:LA]:
                            s()
                    an_drain()
                    for _ in range(5):
                        if pending:
                            pending.pop(0)()
                    if qsb == NQSB - 1 and hp in (1, 3):
                        # last q-block: finalize each 4-head column group as
                        # soon as its normalizers are gathered, so only half
                        # the finalize chain sits on the kernel tail
                        half = hp // 2
                        lnh = rcp.tile([P, SB], dt.float32, tag="lnh",
                                       name="lnh")
                        nc.scalar.activation(
                            lnh[:], sums[:, half * SB:(half + 1) * SB], AF.Ln)
                        rch = rcp.tile([P, SB], dt.bfloat16, tag="rch",
                                       name="rch")
                        with nc.allow_low_precision(reason="bf16 recip"):
                            nc.scalar.activation(rch[:], lnh[:], AF.Exp,
                                                 scale=-1.0)
                        finalize_unit(at_tiles, an_tiles, rch[:], 2 * half)
                        finalize_unit(at_tiles, an_tiles, rch[:], 2 * half + 1)
                if qsb < NQSB - 1:
                    # batched reciprocal for all 8 heads of this q-block, as
                    # exp(-ln(x)) on ACT: both functions live in the
                    # natural_log_exp_and_others table set, so no table
                    # reloads, and it's ~10x cheaper than DVE reciprocal
                    lns = rcp.tile([P, 2 * SB], dt.float32, tag="lns", name="lns")
                    nc.scalar.activation(lns[:], sums[:], AF.Ln)
                    rc = rcp.tile([P, 2 * SB], dt.bfloat16, tag="rc", name="rc")
                    with nc.allow_low_precision(reason="bf16 softmax recip"):
                        nc.scalar.activation(rc[:], lns[:], AF.Exp, scale=-1.0)
                    prev_blk = (qsb, at_tiles, an_tiles, rc)
                else:
                    last_at = at_tiles
            # drain: leftover units of the second-to-last block, then the
            # last block's output projection (its finalize ran per-colgroup)
            for u in pending:
                u()
            for qt in range(4):
                for col in range(2):
                    outproj_unit(NQSB - 1, last_at, qt, col)
    return nc


def _get_program(causal: bool):
    key = bool(causal)
    if key not in _programs:
        _programs[key] = build_program(key)
    return _programs[key]


def kernel(**inputs):
    from concourse.bass_utils import run_bass_kernel_spmd

    xq = np.asarray(inputs["query_sequence"], dtype=np.float32)
    xkv = np.asarray(inputs["key_value_sequence"], dtype=np.float32)
    pmask = np.asarray(inputs["key_value_padding_mask"])
    Wq = np.asarray(inputs["Wq"], dtype=np.float32)
    bq = np.asarray(inputs["bq"], dtype=np.float32)
    Wkv = np.asarray(inputs["Wkv"], dtype=np.float32)
    bkv = np.asarray(inputs["bkv"], dtype=np.float32)
    Wo = np.asarray(inputs["Wo"], dtype=np.float32)
    bo = np.asarray(inputs["bo"], dtype=np.float32)
    causal = bool(np.asarray(inputs["apply_causal_mask"]))

    nc = _get_program(causal)

    Wk_full = Wkv[:, : H * DQ]
    Wv_full = Wkv[:, H * DQ:]
    bk_full = bkv[: H * DQ]
    bv_full = bkv[H * DQ:]

    # causal transition-strip triangle: tri[kv, q'] = 1 if q' >= kv, for the
    # single 128-wide strip of each diagonal block
    kvi = np.arange(P)[:, None]
    qi = np.arange(P)[None, :]
    msk = (qi >= kvi).astype(np.float32).astype(bf16)
    # recip-broadcast selectors, two heads per matmul:
    # variant A (cols 0-127): rows 0/32, variant B (cols 128-255): rows 64/96
    sel = np.zeros((P, 2 * P), np.float32)
    sel[0, 0:64] = 1.0
    sel[32, 64:128] = 1.0
    sel[64, 128:192] = 1.0
    sel[96, 192:256] = 1.0
    sel = sel.astype(bf16)

    in_maps = []
    for c in range(NC):
        b, g = divmod(c, 2)
        hs = slice(g * HL * DQ, (g + 1) * HL * DQ)
        pb = np.where(pmask[b], np.float32(-1e30), np.float32(0.0))
        in_maps.append({
            "xqT": np.ascontiguousarray(xq[b].T).astype(f8),
            "xkvT": np.ascontiguousarray(xkv[b].T).astype(bf16),
            "wq": np.ascontiguousarray(Wq[:, hs] * WS).astype(f8),
            "wk": np.ascontiguousarray(Wk_full[:, hs]).astype(bf16),
            "wv": np.ascontiguousarray(Wv_full[:, hs]).astype(bf16),
            "wo": np.ascontiguousarray(Wo[hs, :]).astype(bf16),
            "bqp": np.ascontiguousarray((bq[hs] * WS).reshape(NHP, P).T),
            "bkp": np.ascontiguousarray(bk_full[hs].reshape(NHP, P).T),
            "pad": np.ascontiguousarray(pb.reshape(NKT, P).T),
            "msk": msk,
            "one64": sel,
        })

    global _last_in_maps
    _last_in_maps = in_maps
    res = run_bass_kernel_spmd(nc, in_maps, core_ids=list(range(NC)))

    host_bias = bo + bv_full @ Wo  # softmax rows sum to 1 -> V-bias is additive
    out = np.empty((B, TQ, DM), np.float32)
    for b in range(B):
        out[b] = res.results[2 * b]["out"] + res.results[2 * b + 1]["out"] + host_bias
    return out


# revision 37
# speedup vs baseline: 1.0216x; 1.0076x over previous
# Multi-head attention (B=4, T=2048, D=1024, H=16, dqk=dv=64) on 8 trn2
# NeuronCores. Sharding: core c -> batch c//2, head-group c%2 (8 heads).
# Each core computes its batch's Q^T/K^T/V projections for its heads,
# causal flash attention with transposed scores (S^T[kv,q]; softmax
# normalizer via a ones-column appended to V), and a partial output
# projection. Host sums the two partials per batch and adds biases.
#
# The Q projection runs in fp8e4m3 DoubleRow mode (2 MACs/cell/cycle):
# weights are host-scaled by 64 so w~N(0,1.3) stays in fp8 normal range,
# making S scale by 64 which the exp scale constant absorbs. K/V and the
# output projection stay bf16: fp8 K too would push output error to 2.4%
# (over the 2e-2 gate); fp8 V/O would inject noise directly on the output.
import numpy as np
import ml_dtypes

B, TQ, TKV, DM, H, DQ, DV = 4, 2048, 2048, 1024, 16, 64, 64
NC = 8          # cores
HL = 8          # heads per core
NHP = HL // 2   # 128-partition head-pair tiles (4)
SB = 512        # q super-block width
NQSB = TQ // SB
NKT = TKV // 128
NDM = DM // 128
NDP = NDM // 2  # fp8 DoubleRow chunk-pairs (4)
P = 128

bf16 = ml_dtypes.bfloat16
f8 = ml_dtypes.float8_e4m3
WS = 64.0       # host-side fp8 weight scale for Q/K

_programs = {}
_last_in_maps = None


def _make_tc_class(tile_mod):
    from concourse.vector_clock import ScopedClock
    import concourse.mybir as mybir

    class TC(tile_mod.TileContext):
        # This toolchain's walrus codegen encodes at most ONE sync wait
        # per instruction. Tile's wait assignment can attach several, so
        # before lowering, peel extra waits off onto standalone
        # InstEventSemaphore instructions placed immediately before the
        # instruction on the same engine (in-order execution makes this
        # semantically identical).
        def _lower_ordered_insts(self, ordered):
            for bb_name, insts in ordered.items():
                out = []
                for inst in insts:
                    si = getattr(inst, "sync_info", None)
                    eng = getattr(inst, "engine", None)
                    if (
                        si is not None
                        and si.on_wait
                        and len(si.on_wait) > 1
                        and eng is not None
                        and eng != mybir.EngineType.Unassigned
                    ):
                        waits = list(si.on_wait)
                        for w in waits[:-1]:
                            ev = mybir.InstEventSemaphore(
                                name=f"I-{self.nc.next_id()}", ins=[], outs=[]
                            )
                            ev.engine = eng
                            ev.sync_info = mybir.SyncInfo(
                                on_wait=[w], on_update=[]
                            )
                            out.append(ev)
                        si.on_wait = waits[-1:]
                    out.append(inst)
                insts[:] = out
            return super()._lower_ordered_insts(ordered)

        # Same 1-wait limit applies to the tail drain; split its waits
        # into standalone wait instructions.
        def _drain_and_barrier(self, tick_clock, wait_clock):
            drain_inst = self.nc.sync.drain()
            wait_clock.add_sem_waits(
                drain_inst.ins, ScopedClock({None: tick_clock.global_clock})
            )
            si = drain_inst.ins.sync_info
            waits = list(si.on_wait) if si and si.on_wait else []
            if len(waits) > 1:
                si.on_wait = waits[:1]
                name2sem = {}
                for s in self.sems.allocated().values():
                    name2sem[getattr(s, "name", None) or str(s)] = s
                for w in waits[1:]:
                    self.nc.sync.wait_ge(name2sem[w.ant_name], w.wait_value)
            self.nc.all_engine_barrier()
            popped = self.nc._tile_sem_poison_stack.pop()
            assert popped is self._sem_poison
            self.nc.clear_and_free_semaphores(list(self.sems.allocated().values()))
            self.nc.all_engine_barrier()

    return TC


def build_program(causal: bool):
    import concourse.bass as bass
    import concourse.mybir as mybir
    import concourse.tile as tile

    dt = mybir.dt
    AF = mybir.ActivationFunctionType
    DR = mybir.MatmulPerfMode.DoubleRow
    TC = _make_tc_class(tile)

    nc = bass.Bass("TRN2", target_bir_lowering=False, debug=False, num_devices=NC)

    xqT = nc.dram_tensor("xqT", [DM, TQ], dt.float8e4, kind="ExternalInput")
    xkvT = nc.dram_tensor("xkvT", [DM, TKV], dt.bfloat16, kind="ExternalInput")
    wq_d = nc.dram_tensor("wq", [DM, HL * DQ], dt.float8e4, kind="ExternalInput")
    wk_d = nc.dram_tensor("wk", [DM, HL * DQ], dt.bfloat16, kind="ExternalInput")
    wv_d = nc.dram_tensor("wv", [DM, HL * DV], dt.bfloat16, kind="ExternalInput")
    wo_d = nc.dram_tensor("wo", [HL * DV, DM], dt.bfloat16, kind="ExternalInput")
    bq_d = nc.dram_tensor("bqp", [P, NHP], dt.float32, kind="ExternalInput")
    bk_d = nc.dram_tensor("bkp", [P, NHP], dt.float32, kind="ExternalInput")
    pad_d = nc.dram_tensor("pad", [P, NKT], dt.float32, kind="ExternalInput")
    msk_d = nc.dram_tensor("msk", [P, P], dt.bfloat16, kind="ExternalInput")
    one_d = nc.dram_tensor("one64", [P, 2 * P], dt.bfloat16, kind="ExternalInput")
    out_d = nc.dram_tensor("out", [TQ, DM], dt.float32, kind="ExternalOutput")

    # exp(scale * S' + pad): S' = (64Q)·K, true logits need /sqrt(64)
    EXP_SCALE = 0.125 / WS

    with TC(nc) as tc:
        with (
            tc.tile_pool(name="res", bufs=1) as res,
            tc.tile_pool(name="xp", bufs=8) as xp,
            tc.tile_pool(name="ptp", bufs=4) as ptp,
            tc.tile_pool(name="atp", bufs=2) as atp,
            tc.tile_pool(name="rcp", bufs=2) as rcp,
            tc.tile_pool(name="ps_proj", bufs=2, space="PSUM") as ps_proj,
            tc.tile_pool(name="ps_s", bufs=2, space="PSUM") as ps_s,
            tc.tile_pool(name="ps_at", bufs=2, space="PSUM") as ps_at,
        ):
            # ---- Q-projection critical path first: wq + bq + xqT ----
            # one trigger per tensor: each dma_start costs ~600ns of serial
            # issue time on the Sync engine, so bulk loads use a single wide
            # 3D-AP DMA with dm-chunks side by side in the free dim
            wq_t = res.tile([P, NDM * HL * DQ], dt.float8e4, tag="wq", name="wq")
            nc.sync.dma_start(
                wq_t[:].rearrange("p (k m) -> p k m", k=NDM),
                wq_d.ap().rearrange("(k p) m -> p k m", p=P))

            def wq_pair(cp, hp):
                # fp8 DoubleRow lhsT [128, ko=2, 128]: ko = second 128-row
                # half of the 256-row dm chunk-pair
                return wq_t[:, 2 * cp * HL * DQ:(2 * cp + 2) * HL * DQ].rearrange(
                    "p (ko m) -> p ko m", ko=2)[:, :, hp * P:(hp + 1) * P]

            bq_t = res.tile([P, NHP], dt.float32, tag="bq", name="bq_t")
            nc.sync.dma_start(bq_t[:], bq_d.ap()[:, :])

            # ---- Q^T projection, per q super-block (chunked xq) ----
            qT = [res.tile([P, TQ], dt.bfloat16, tag=f"qT{hp}", name=f"qT{hp}") for hp in range(NHP)]
            xq_cache = {}

            def load_xq(qsb):
                if qsb in xq_cache:
                    return xq_cache[qsb]
                t = xp.tile([P, NDM * SB], dt.float8e4, tag="xq", name="xq",
                            bufs=4)
                nc.sync.dma_start(
                    t[:].rearrange("p (k n) -> p k n", k=NDM),
                    xqT.ap()[:, qsb * SB:(qsb + 1) * SB].rearrange(
                        "(k p) n -> p k n", p=P))
                xq_cache[qsb] = t
                return t

            def q_units(qsb):
                def unit(hp):
                    ps = ps_proj.tile([P, SB], dt.float32, tag="pp", name="pp")
                    xqt = load_xq(qsb)
                    for k in range(NDP):
                        nc.tensor.matmul(
                            ps[:],
                            wq_pair(k, hp),
                            xqt[:, 2 * k * SB:(2 * k + 2) * SB].rearrange(
                                "p (ko n) -> p ko n", ko=2),
                            start=(k == 0), stop=(k == NDP - 1),
                            perf_mode=DR,
                        )
                    with nc.allow_low_precision(reason="bf16 Q"):
                        nc.vector.tensor_scalar_add(
                            qT[hp][:, qsb * SB:(qsb + 1) * SB], ps[:],
                            bq_t[:, hp:hp + 1],
                        )
                return [lambda hp=hp: unit(hp) for hp in range(NHP)]

            load_xq(0)  # qsb0 activations right behind wq on the DMA queue
            xkv_cache = {}

            def load_xkv(nj):
                if nj in xkv_cache:
                    return xkv_cache[nj]
                t = xp.tile([P, NDM * SB], dt.bfloat16, tag="xkv", name="xkv",
                            bufs=4)
                nc.sync.dma_start(
                    t[:].rearrange("p (k n) -> p k n", k=NDM),
                    xkvT.ap()[:, nj * SB:(nj + 1) * SB].rearrange(
                        "(k p) n -> p k n", p=P))
                xkv_cache[nj] = t
                return t

            # the 4MB xkv block gates the K/V projections -> trigger it
            # before the weights so its transfer leads the queue
            load_xkv(0)
            wk_t = res.tile([P, NDM * HL * DQ], dt.bfloat16, tag="wk", name="wk")
            nc.sync.dma_start(
                wk_t[:].rearrange("p (k m) -> p k m", k=NDM),
                wk_d.ap().rearrange("(k p) m -> p k m", p=P))
            wv_t = res.tile([P, NDM * HL * DV], dt.bfloat16, tag="wv", name="wv")
            nc.sync.dma_start(
                wv_t[:].rearrange("p (k m) -> p k m", k=NDM),
                wv_d.ap().rearrange("(k p) m -> p k m", p=P))
            bk_t = res.tile([P, NHP], dt.float32, tag="bk", name="bk_t")
            nc.sync.dma_start(bk_t[:], bk_d.ap()[:, :])
            pad_t = res.tile([P, NKT], dt.float32, tag="pad", name="pad_t")
            nc.sync.dma_start(pad_t[:], pad_d.ap()[:, :])
            msk_t = res.tile([P, P], dt.bfloat16, tag="msk", name="msk_t")
            nc.sync.dma_start(msk_t[:], msk_d.ap()[:, :])
            sel_t = res.tile([P, 2 * P], dt.bfloat16, tag="sel", name="sel_t")
            nc.sync.dma_start(sel_t[:], one_d.ap()[:, :])
            wo_t = res.tile([P, NHP * DM], dt.bfloat16, tag="wo", name="wo")
            nc.sync.dma_start(
                wo_t[:].rearrange("p (k m) -> p k m", k=NHP),
                wo_d.ap().rearrange("(k p) m -> p k m", p=P))

            # ---- K^T and V projections (kv-chunk major so attention can
            # start early). V layout: per kv tile [128, 8 heads x 65]
            # (64 V cols + 1 ones col per head) for the PV+rowsum matmul. ----
            kT = [res.tile([P, TKV], dt.bfloat16, tag=f"kT{hp}", name=f"kT{hp}") for hp in range(NHP)]
            v_t = [res.tile([P, HL * 65], dt.bfloat16, tag=f"v{vt}", name=f"v{vt}") for vt in range(NKT)]
            def kv_units(nj):
                def k_unit(hp):
                    ps = ps_proj.tile([P, SB], dt.float32, tag="pp", name="pp")
                    xkt = load_xkv(nj)
                    for k in range(NDM):
                        nc.tensor.matmul(
                            ps[:],
                            wk_t[:, k * HL * DQ + hp * P:k * HL * DQ + (hp + 1) * P],
                            xkt[:, k * SB:(k + 1) * SB],
                            start=(k == 0), stop=(k == NDM - 1),
                        )
                    with nc.allow_low_precision(reason="bf16 K"):
                        nc.vector.tensor_scalar_add(
                            kT[hp][:, nj * SB:(nj + 1) * SB], ps[:],
                            bk_t[:, hp:hp + 1],
                        )

                def v_unit(vt):
                    ps = ps_proj.tile([P, SB], dt.float32, tag="pp", name="pp")
                    xkt = load_xkv(vt // 4)
                    for k in range(NDM):
                        nc.tensor.matmul(
                            ps[:],
                            xkt[:, k * SB + (vt % 4) * P:k * SB + (vt % 4 + 1) * P],
                            wv_t[:, k * HL * DV:(k + 1) * HL * DV],
                            start=(k == 0), stop=(k == NDM - 1),
                        )
                    vtile = v_t[vt]
                    nc.vector.tensor_copy(
                        vtile[:].rearrange("p (h d) -> p h d", d=65)[:, :, 0:64],
                        ps[:].rearrange("p (h d) -> p h d", d=64),
                    )
                    nc.gpsimd.memset(
                        vtile[:].rearrange("p (h d) -> p h d", d=65)[:, :, 64:65], 1.0
                    )
                units = [lambda hp=hp: k_unit(hp) for hp in range(NHP)]
                units += [lambda vt=vt: v_unit(vt) for vt in range(nj * 4, nj * 4 + 4)]
                return units

            # ---- attention + output projection, per q super-block ----
            def finalize_unit(at_tiles, an_tiles, rc_ap, hp):
                # broadcast the pair's recip rows across 2x64 partitions
                # via PE: sel variant (hp%2) places head 2hp's recip on
                # rows 0-63 and head 2hp+1's on rows 64-127
                bc = ps_proj.tile([P, SB], dt.float32, tag="pp", name="bc")
                v = hp % 2
                nc.tensor.matmul(
                    bc[:], sel_t[:, v * P:(v + 1) * P],
                    rc_ap,
                    start=True, stop=True,
                )
                nc.vector.tensor_mul(
                    at_tiles[hp][0:64, :], an_tiles[2 * hp][0:64, :],
                    bc[0:64, :]
                )
                nc.vector.tensor_mul(
                    at_tiles[hp][64:128, :], an_tiles[2 * hp + 1][0:64, :],
                    bc[64:128, :]
                )

            def outproj_unit(qsb, at_tiles, qt, col):
                ps = ps_proj.tile([P, SB], dt.float32, tag="pp", name="pp")
                for hp in range(NHP):
                    nc.tensor.matmul(
                        ps[:],
                        at_tiles[hp][:, qt * P:(qt + 1) * P],
                        wo_t[:, hp * DM + col * SB:hp * DM + (col + 1) * SB],
                        start=(hp == 0), stop=(hp == NHP - 1),
                    )
                ost = rcp.tile([P, SB], dt.float32, tag="ost", name="ost", bufs=3)
                nc.vector.tensor_copy(ost[:], ps[:])
                r0 = qsb * SB + qt * P
                nc.sync.dma_start(
                    out_d.ap()[r0:r0 + P, col * SB:(col + 1) * SB], ost[:]
                )

            def blk_units(qsb, at_tiles, an_tiles, rc):
                # finalize + output projection for a finished q block, as
                # interleavable units (finalize first: outproj reads at_tiles)
                units = [
                    lambda hp=hp: finalize_unit(
                        at_tiles, an_tiles,
                        rc[:, (hp // 2) * SB:(hp // 2 + 1) * SB], hp)
                    for hp in range(NHP)
                ]
                units += [
                    lambda qt=qt, col=col: outproj_unit(qsb, at_tiles, qt, col)
                    for qt in range(4) for col in range(2)
                ]
                return units

            prev_blk = None
            pending = q_units(0) + kv_units(0)
            for qsb in range(NQSB):
                # flush any units for THIS block not yet emitted
                for u in pending:
                    u()
                # next block's projection units AND the previous block's
                # finalize+outproj get interleaved between this block's head
                # pairs so the in-order PE stream always has independent
                # matmul work while ACT chews on exps
                pending = (
                    q_units(qsb + 1) + kv_units(qsb + 1)
                    if qsb + 1 < NQSB else []
                )
                if prev_blk is not None:
                    pending = pending + blk_units(*prev_blk)
                at_tiles = [
                    atp.tile([P, SB], dt.bfloat16, tag=f"attnT{hp}", name=f"attnT{hp}")
                    for hp in range(NHP)
                ]
                kt_max = 4 * qsb + 4 if causal else NKT
                # normalizer rows gathered at partitions {0,32,64,96} x 2
                # column groups (DVE partition offsets must be 32-aligned);
                # filler 1.0 keeps the reciprocal finite on unused rows
                sums = rcp.tile([P, 2 * SB], dt.bfloat16, tag="sums", name="sums")
                nc.gpsimd.memset(sums[:], 1.0)
                an_tiles = []

                def build_hp(hp):
                    # the pair's S matmuls target disjoint PE row groups
                    # (0-63 / 64-127) and disjoint banks of one wide PSUM
                    # tile, so they run concurrently and one wide exp
                    # covers both heads
                    st = {"aps2": None, "pv_q": []}

                    def emit_pv(kt, pt, c0):
                        if st["aps2"] is None:
                            # lazy: allocated at first PV emission, which the
                            # schedule places after the previous pair's drain
                            st["aps2"] = [
                                ps_at.tile([65, SB], dt.float32, tag="at",
                                           name="at")
                                for _ in range(2)
                            ]
                        for e in range(2):
                            h = 2 * hp + e
                            nc.tensor.matmul(
                                st["aps2"][e][:, c0:SB],
                                v_t[kt][:, h * 65:h * 65 + 65],
                                pt[:, e * SB + c0:(e + 1) * SB],
                                start=(kt == 0), stop=(kt == kt_max - 1),
                            )

                    # PV for tile kt is emitted AFTER tile kt+2's S matmuls,
                    # so the in-order PE queue never waits on exp(kt): two
                    # more S pairs run while ACT produces pt(kt)
                    def kt_step(kt):
                        # diagonal blocks (j>=1): only q columns >= 128*j can
                        # be unmasked -> trim the left columns entirely
                        j = kt - 4 * qsb if causal else -1
                        c0 = 128 * j if j > 0 else 0
                        sps = ps_s.tile([P, 2 * SB], dt.float32, tag="s",
                                        name="s")
                        for e in range(2):
                            off = e * 64
                            nc.tensor.matmul(
                                sps[:, e * SB + c0:(e + 1) * SB],
                                kT[hp][off:off + 64, kt * P:(kt + 1) * P],
                                qT[hp][off:off + 64,
                                       qsb * SB + c0:(qsb + 1) * SB],
                                start=True, stop=True,
                            )
                        pt = ptp.tile([P, 2 * SB], dt.bfloat16, tag="pT",
                                      name="pT")
                        nc.scalar.activation(
                            pt[:].rearrange("p (e q) -> p e q", e=2)[:, :, c0:SB],
                            sps[:].rearrange("p (e q) -> p e q", e=2)[:, :, c0:SB],
                            AF.Exp,
                            scale=EXP_SCALE, bias=pad_t[:, kt:kt + 1],
                        )
                        if causal and j >= 0:
                            # only the 128-wide transition strip needs the
                            # triangle; columns beyond it are fully unmasked
                            for e in range(2):
                                nc.vector.tensor_mul(
                                    pt[:, e * SB + c0:e * SB + c0 + P],
                                    pt[:, e * SB + c0:e * SB + c0 + P],
                                    msk_t[:],
                                )
                        st["pv_q"].append((kt, pt, c0))
                        if len(st["pv_q"]) > 2:
                            emit_pv(*st["pv_q"].pop(0))
                        # drip one boundary-work unit into the kt stream every
                        # few steps: the S+PV pair leaves ~200ns of PE slack
                        # per ACT-paced step, so a small chain here converts
                        # head-pair-boundary burst time into overlapped time
                        if kt % 4 == 3 and kt + 1 < kt_max and pending:
                            pending.pop(0)()

                    def pv_flush():
                        for args in st["pv_q"]:
                            emit_pv(*args)
                        st["pv_q"] = []

                    def an_drain():
                        for e in range(2):
                            h = 2 * hp + e
                            # stage numerator + normalizer row to SBUF, free
                            # psum; the normalizer row rides along and is
                            # gathered into the sums tile by a DMA engine --
                            # except the very last pair, whose gather sits on
                            # the kernel tail: DVE beats the ~1.7us DMA
                            # launch latency there
                            an = rcp.tile([65, SB], dt.bfloat16, tag=f"an{h}",
                                          name=f"an{h}")
                            nc.vector.tensor_copy(an[:], st["aps2"][e][0:65, :])
                            r, cg = 32 * (h % 4), (h // 4) * SB
                            nc.sync.dma_start(
                                sums[r:r + 1, cg:cg + SB], an[64:65, :]
                            )
                            an_tiles.append(an)

                    steps = [lambda kt=kt: kt_step(kt) for kt in range(kt_max)]
                    return steps, pv_flush, an_drain

                # measured: emitting the next pair's first S/exp steps before
                # this pair's drain (cross-pair lookahead) ran slower — the
                # next pair's S stalls on PSUM buffers and blocks the ready
                # drain work behind it in the in-order queue. Keep LA=0.
                LA = 0
                hps = [build_hp(hp) for hp in range(NHP)]
                for hp in range(NHP):
                    steps, pv_flush, an_drain = hps[hp]
                    for s in steps[LA if hp > 0 else 0:]:
                        s()
                    pv_flush()
                    if LA and hp + 1 < NHP:
                        for s in hps[hp + 1][0][:LA]:
                            s()
                    an_drain()
                    for _ in range(5):
                        if pending:
                            pending.pop(0)()
                    if qsb == NQSB - 1 and hp in (1, 3):
                        # last q-block: finalize each 4-head column group as
                        # soon as its normalizers are gathered, so only half
                        # the finalize chain sits on the kernel tail
                        half = hp // 2
                        lnh = rcp.tile([P, SB], dt.float32, tag="lnh",
                                       name="lnh")
                        nc.scalar.activation(
                            lnh[:], sums[:, half * SB:(half + 1) * SB], AF.Ln)
                        rch = rcp.tile([P, SB], dt.bfloat16, tag="rch",
                                       name="rch")
                        with nc.allow_low_precision(reason="bf16 recip"):
                            nc.scalar.activation(rch[:], lnh[:], AF.Exp,
                                                 scale=-1.0)
                        finalize_unit(at_tiles, an_tiles, rch[:], 2 * half)
                        finalize_unit(at_tiles, an_tiles, rch[:], 2 * half + 1)
                if qsb < NQSB - 1:
                    # batched reciprocal for all 8 heads of this q-block, as
                    # exp(-ln(x)) on ACT: both functions live in the
                    # natural_log_exp_and_others table set, so no table
                    # reloads, and it's ~10x cheaper than DVE reciprocal
                    lns = rcp.tile([P, 2 * SB], dt.float32, tag="lns", name="lns")
                    nc.scalar.activation(lns[:], sums[:], AF.Ln)
                    rc = rcp.tile([P, 2 * SB], dt.bfloat16, tag="rc", name="rc")
                    with nc.allow_low_precision(reason="bf16 softmax recip"):
                        nc.scalar.activation(rc[:], lns[:], AF.Exp, scale=-1.0)
                    prev_blk = (qsb, at_tiles, an_tiles, rc)
                else:
                    last_at = at_tiles
            # drain: leftover units of the second-to-last block, then the
            # last block's output projection (its finalize ran per-colgroup)
            for u in pending:
                u()
            for qt in range(4):
                for col in range(2):
                    outproj_unit(NQSB - 1, last_at, qt, col)
    return nc


def _get_program(causal: bool):
    key = bool(causal)
    if key not in _programs:
        _programs[key] = build_program(key)
    return _programs[key]


def kernel(**inputs):
    from concourse.bass_utils import run_bass_kernel_spmd

    xq = np.asarray(inputs["query_sequence"], dtype=np.float32)
    xkv = np.asarray(inputs["key_value_sequence"], dtype=np.float32)
    pmask = np.asarray(inputs["key_value_padding_mask"])
    Wq = np.asarray(inputs["Wq"], dtype=np.float32)
    bq = np.asarray(inputs["bq"], dtype=np.float32)
    Wkv = np.asarray(inputs["Wkv"], dtype=np.float32)
    bkv = np.asarray(inputs["bkv"], dtype=np.float32)
    Wo = np.asarray(inputs["Wo"], dtype=np.float32)
    bo = np.asarray(inputs["bo"], dtype=np.float32)
    causal = bool(np.asarray(inputs["apply_causal_mask"]))

    nc = _get_program(causal)

    Wk_full = Wkv[:, : H * DQ]
    Wv_full = Wkv[:, H * DQ:]
    bk_full = bkv[: H * DQ]
    bv_full = bkv[H * DQ:]

    # causal transition-strip triangle: tri[kv, q'] = 1 if q' >= kv, for the
    # single 128-wide strip of each diagonal block
    kvi = np.arange(P)[:, None]
    qi = np.arange(P)[None, :]
    msk = (qi >= kvi).astype(np.float32).astype(bf16)
    # recip-broadcast selectors, two heads per matmul:
    # variant A (cols 0-127): rows 0/32, variant B (cols 128-255): rows 64/96
    sel = np.zeros((P, 2 * P), np.float32)
    sel[0, 0:64] = 1.0
    sel[32, 64:128] = 1.0
    sel[64, 128:192] = 1.0
    sel[96, 192:256] = 1.0
    sel = sel.astype(bf16)

    in_maps = []
    for c in range(NC):
        b, g = divmod(c, 2)
        hs = slice(g * HL * DQ, (g + 1) * HL * DQ)
        pb = np.where(pmask[b], np.float32(-1e30), np.float32(0.0))
        in_maps.append({
            "xqT": np.ascontiguousarray(xq[b].T).astype(f8),
            "xkvT": np.ascontiguousarray(xkv[b].T).astype(bf16),
            "wq": np.ascontiguousarray(Wq[:, hs] * WS).astype(f8),
            "wk": np.ascontiguousarray(Wk_full[:, hs]).astype(bf16),
            "wv": np.ascontiguousarray(Wv_full[:, hs]).astype(bf16),
            "wo": np.ascontiguousarray(Wo[hs, :]).astype(bf16),
            "bqp": np.ascontiguousarray((bq[hs] * WS).reshape(NHP, P).T),
            "bkp": np.ascontiguousarray(bk_full[hs].reshape(NHP, P).T),
            "pad": np.ascontiguousarray(pb.reshape(NKT, P).T),
            "msk": msk,
            "one64": sel,
        })

    global _last_in_maps
    _last_in_maps = in_maps
    res = run_bass_kernel_spmd(nc, in_maps, core_ids=list(range(NC)))

    host_bias = bo + bv_full @ Wo  # softmax rows sum to 1 -> V-bias is additive
    out = np.empty((B, TQ, DM), np.float32)
    for b in range(B):
        out[b] = res.results[2 * b]["out"] + res.results[2 * b + 1]["out"] + host_bias
    return out
